# revision 21
# baseline (speedup 1.0000x reference)
"""Trainium2 Bass kernel for a pre-norm transformer encoder layer.

Problem shapes: B=2, S=4096, E=512, H=8 (Dh=64), FF=2048, fp32 I/O.

Sharding (zero cross-core communication): core c handles batch b=c//4 and
query rows qr=(c%4)*1024.  Each core redundantly computes LN1 + K/V for its
batch's full 4096 tokens, then attention for all 8 heads over its own 1024
queries, then Wo / LN2 / FFN token-parallel.  The per-core token stream is
rotated so the core's queries are tokens 0..1023 — attention is invariant
to key/value ordering, so one SPMD program serves all cores.

Engine strategy:
- softmax exp alternates between ScalarE (native Exp LUT -> fp8e5m2) and
  VectorE (Schraudolph bit trick: e5m2 bitpattern = round(4/ln2 * s/8 +
  60 - 0.23) via one tensor_scalar into an int8 view).  The ~6-12%
  sawtooth washes out after softmax averaging over ~1.5k effective keys.
- A@V runs in fp8 DoubleRow mode: V in e4m3 padded to 80 columns (ones at
  col 64 produce the softmax denominators; 65..79 zero), two k-tiles per
  matmul -> half the PE slots of the bf16 version.
- 1/den and LN1's 1/std come from a VectorE log bit-trick followed by a
  ScalarE exp (these errors only perturb the small attention branch);
  LN2's 1/std (which scales the large FFN branch) uses an exact rsqrt
  bit-trick + 2 Newton iterations on VectorE.  No Ln/Sqrt activations ->
  the whole kernel uses a single ACT table set (exp/relu/copy).
"""

import sys

if "/opt/trn_rl_repo" not in sys.path:
    sys.path.insert(0, "/opt/trn_rl_repo")

from contextlib import ExitStack

import ml_dtypes
import numpy as np

import concourse.bacc as bacc
import concourse.tile as tile
from concourse import mybir
from concourse.bass_utils import run_bass_kernel_spmd

B, S, E, H, Dh, FF = 2, 4096, 512, 8, 64, 2048
NCORES = 8
QPC = 1024  # queries per core
F32 = mybir.dt.float32
BF16 = mybir.dt.bfloat16
I8 = mybir.dt.int8
I32 = mybir.dt.int32
FP8 = mybir.dt.float8e4
FP8E5 = mybir.dt.float8e5
AF = mybir.ActivationFunctionType
ALU = mybir.AluOpType
DR = mybir.MatmulPerfMode.DoubleRow
P = 128
NKT = S // P   # 32 k-tiles
NKP = NKT // 2  # 16 k-tile pairs
VW = 80        # per-head V columns: 64 data + ones@64 + zero pad to 80

LN2_ = float(np.log(2.0))
A_SCH = 4.0 / (8.0 * LN2_)   # e5m2 Schraudolph slope (1/sqrt(Dh) folded)
B_SCH = 4.0 * 15.0 - 0.23
KLOG = 8.262958405e-8        # ln2 / 2^23
B0 = 1064866805.0            # fp32 log bit-trick bias (mu=0.0430)
K1 = float(E) / (E - 1)      # unbiased-variance correction
C_LN1 = B0 - float(np.log(K1)) / KLOG
RSQ_C = float(0x5F3759DF)

_CACHE = {}


def _emit(nc, tc, ext):
    es = ExitStack()
    with es:
        persist = es.enter_context(tc.tile_pool(name="persist", bufs=1))
        p34 = es.enter_context(tc.tile_pool(name="p34", bufs=1))
        st2 = es.enter_context(tc.tile_pool(name="st2", bufs=4))
        kqv_cm = tc.tile_pool(name="kqv", bufs=1)
        kqv = kqv_cm.__enter__()

        xq_sb = persist.tile([P, 8, E], F32)
        x2_sb = persist.tile([P, 8, E], F32)
        ctxT = persist.tile([P, 4, QPC], BF16)
        bq_sb = persist.tile([P, 4], F32)
        b1_sb = persist.tile([P, 16], F32)
        b2_sb = persist.tile([P, E], F32)
        ln_sc = persist.tile([P, 4], F32)  # alpha1,bias1,alpha2,bias2 bcast
        ident = persist.tile([P, P], BF16)
        xn2T = p34.tile([P, 4, QPC], BF16)
        xn2 = p34.tile([P, 4, E], BF16)
        wo_sb = p34.tile([P, 4, E], BF16)
        w1_sb = persist.tile([P, 4, FF], BF16)
        w2_sb = persist.tile([P, 16, E], BF16)

        kT = kqv.tile([P, 4, S], BF16)
        qT = kqv.tile([P, 4, QPC], BF16)
        wq_sb = kqv.tile([P, 4, E], FP8)
        wk_sb = kqv.tile([P, 4, E], FP8)
        xnT = kqv.tile([P, 4, S], FP8)
        vD = kqv.tile([P, NKP, H, 2, VW], FP8)

        # ---- setup loads -------------------------------------------------
        nc.sync.dma_start(out=bq_sb, in_=ext["bq"][:])
        nc.gpsimd.dma_start(out=b2_sb, in_=ext["b2"][:].unsqueeze(0).to_broadcast((P, E)))
        for i, nm in enumerate(["a1", "c1", "a2", "c2"]):
            nc.gpsimd.dma_start(out=ln_sc[:, i : i + 1], in_=ext[nm][:].to_broadcast((P, 1)))
        nc.sync.dma_start(out=ident, in_=ext["ident"][:])
        nc.vector.memset(vD[:, :, :, :, Dh : Dh + 1], 1.0)
        nc.vector.memset(vD[:, :, :, :, Dh + 1 :], 0.0)

        # ---- phase 0/1: LN1, transpose, QKV projections ------------------
        with tc.tile_pool(name="wqkv", bufs=1) as wp, \
             tc.tile_pool(name="xn_s", bufs=3) as xnp, \
             tc.tile_pool(name="x_s", bufs=24) as xs, \
             tc.tile_pool(name="st1", bufs=6) as stp, \
             tc.tile_pool(name="ps1", bufs=4, space="PSUM") as ps1:

            wv_sb = wp.tile([P, 4, E], FP8)

            xtiles = []
            for i in range(NKT):
                xt = xs.tile([P, E], BF16)
                eng = nc.sync if i < 12 else nc.gpsimd
                eng.dma_start(out=xt, in_=ext["xb"][P * i : P * (i + 1), :])
                xtiles.append(xt)
                if i == 3:
                    nc.sync.dma_start(out=wk_sb, in_=ext["wk"][:])
                    nc.sync.dma_start(out=wv_sb, in_=ext["wv"][:])
                    nc.sync.dma_start(out=wq_sb, in_=ext["wq"][:])
                if i == 11:
                    nc.sync.dma_start(out=xq_sb, in_=ext["xq"][:])
                    nc.sync.dma_start(out=b1_sb, in_=ext["b1"][:])
                    nc.gpsimd.dma_start(out=wo_sb, in_=ext["wo"][:])
            # prefetch FFN weights once all x tiles are queued: done long
            # before the tail, without delaying the LayerNorm input stream
            nc.gpsimd.dma_start(out=w1_sb, in_=ext["w1"][:])
            nc.gpsimd.dma_start(out=w2_sb, in_=ext["w2"][:])

            def kq_group(c, tb, w_sb, dstT, bias, pool):
                # fp8 DoubleRow: two k-tile-paired matmuls instead of four;
                # PSUM evacuation on ScalarE (VectorE is the busier engine)
                acc = pool.tile([P, E], F32, tag="po")
                for p in range(2):
                    nc.tensor.matmul(acc, lhsT=w_sb[:, 2 * p : 2 * p + 2, P * c : P * (c + 1)],
                                     rhs=xnT[:, 2 * p : 2 * p + 2, 512 * tb : 512 * (tb + 1)],
                                     start=(p == 0), stop=(p == 1), perf_mode=DR)
                dst = dstT[:, c, 512 * tb : 512 * (tb + 1)]
                if bias is None:
                    nc.scalar.copy(out=dst, in_=acc)
                else:
                    nc.scalar.activation(out=dst, in_=acc, func=AF.Identity,
                                         bias=bias[:, c : c + 1])

            for g in range(8):
                mv = stp.tile([P, 4, 2], F32, tag="mv")
                for j in range(4):
                    i = 4 * g + j
                    st6 = stp.tile([P, 6], F32, tag="st6")
                    nc.vector.bn_stats(out=st6, in_=xtiles[i])
                    nc.vector.bn_aggr(out=mv[:, j, :], in_=st6)
                sc = stp.tile([P, 4], F32, tag="sc")
                tt = stp.tile([P, 4], F32, tag="tt")
                y1 = stp.tile([P, 4], F32, tag="y1")
                z1 = stp.tile([P, 4], F32, tag="z1")
                # 1/std(ddof=1): rsqrt bit trick + 1 Newton (max ~0.2% err)
                var1 = mv[:, :, 1]
                nc.vector.tensor_scalar(out=y1.bitcast(I32), in0=var1.bitcast(I32),
                                        scalar1=-0.5, scalar2=RSQ_C,
                                        op0=ALU.mult, op1=ALU.add)
                nc.vector.tensor_mul(z1, y1, y1)
                nc.vector.tensor_mul(z1, z1, var1)
                nc.vector.tensor_scalar(out=z1, in0=z1, scalar1=-0.5, scalar2=1.5,
                                        op0=ALU.mult, op1=ALU.add)
                nc.vector.tensor_mul(y1, z1, y1)
                # sc = rsqrt(var)*alpha1/sqrt(K1)
                nc.vector.tensor_scalar(out=sc, in0=y1, scalar1=ln_sc[:, 0:1],
                                        scalar2=float(1.0 / np.sqrt(K1)),
                                        op0=ALU.mult, op1=ALU.mult)
                nc.vector.tensor_mul(tt, mv[:, :, 0], sc)
                nc.vector.tensor_scalar(out=tt, in0=tt, scalar1=ln_sc[:, 1:2], scalar2=None, op0=ALU.subtract)
                for j in range(4):
                    i = 4 * g + j
                    xnt = xnp.tile([P, E], BF16)
                    nc.vector.tensor_scalar(out=xnt, in0=xtiles[i], scalar1=sc[:, j : j + 1],
                                            scalar2=tt[:, j : j + 1], op0=ALU.mult, op1=ALU.subtract)
                    ptp = ps1.tile([P, 4, P], BF16, tag="ptp")
                    for e in range(4):
                        nc.tensor.transpose(ptp[:, e, :], xnt[:, P * e : P * (e + 1)], ident)
                    nc.scalar.copy(out=xnT[:, :, P * i : P * (i + 1)], in_=ptp)
                # this token block is transposed: K/Q chunk-0 + its V k-tiles
                kq_group(0, g, wk_sb, kT, None, ps1)
                if g < 2:
                    kq_group(0, g, wq_sb, qT, bq_sb, ps1)
                for j in range(4):
                    kt = 4 * g + j
                    acc = ps1.tile([P, E], F32, tag="po")
                    for p in range(2):
                        nc.tensor.matmul(acc, lhsT=xnT[:, 2 * p : 2 * p + 2, P * kt : P * (kt + 1)],
                                         rhs=wv_sb[:, 2 * p : 2 * p + 2, :],
                                         start=(p == 0), stop=(p == 1), perf_mode=DR)
                    dst = vD[:, kt // 2, :, kt % 2, 0:Dh]
                    nc.scalar.copy(out=dst, in_=acc.rearrange("p (h d) -> p h d", d=Dh))

        # ---- phase 2: attention (+ overlapped Wo/LN2 per query half) ----
        with tc.tile_pool(name="exp_p", bufs=4) as expp, \
             tc.tile_pool(name="rs_p", bufs=4) as rsp, \
             tc.tile_pool(name="ps_sa", bufs=1, space="PSUM") as pssa, \
             tc.tile_pool(name="ps_sb", bufs=1, space="PSUM") as pssb, \
             tc.tile_pool(name="ps_c", bufs=2, space="PSUM") as psc, \
             tc.tile_pool(name="ps_o", bufs=2, space="PSUM") as pso:
            from collections import deque
            fillers = deque()
            for c in range(1, 4):
                for tb in range(8):
                    fillers.append((c, tb, wk_sb, kT, None))
                for tb in range(2):
                    fillers.append((c, tb, wq_sb, qT, bq_sb))
            def av(pcs, hp2, pep, pkp):
                for par in range(2):
                    nc.tensor.matmul(pcs[par][:, :],
                                     lhsT=vD[:, pkp, 2 * hp2 + par, :, :],
                                     rhs=pep[:, :, par, :],
                                     start=(pkp == 0), stop=(pkp == NKP - 1),
                                     perf_mode=DR)

            def flush_norm(pcs, ch2, qo2):
                for par in range(2):
                    h = 2 * ch2 + par
                    r0 = 64 * (h % 2)
                    # 1/den = exp(-ln(den)), ln via bit trick on DVE
                    lnd = rsp.tile([1, 512], F32, tag="lnd")
                    rs = rsp.tile([1, 512], F32, tag="rs")
                    nc.vector.tensor_scalar(out=lnd, in0=pcs[par][Dh : Dh + 1, :].bitcast(I32),
                                            scalar1=B0, scalar2=-KLOG,
                                            op0=ALU.subtract, op1=ALU.mult)
                    nc.scalar.activation(out=rs, in_=lnd, func=AF.Exp)
                    bc = rsp.tile([64, 512], F32, tag="bc")
                    nc.gpsimd.partition_broadcast(bc, rs)
                    nc.vector.tensor_mul(ctxT[r0 : r0 + 64, ch2, qo2 : qo2 + 512],
                                         pcs[par][0:Dh, :], bc)

            def flush_wo(qc2):
                # Wo + residual + LN2 + xn2 transpose for one query half
                mv2 = st2.tile([P, 4, 2], F32, tag="mv")
                for jq in range(4):
                    qb = 4 * qc2 + jq
                    po = pso.tile([P, E], F32, tag="po")
                    for c in range(4):
                        nc.tensor.matmul(po, lhsT=ctxT[:, c, P * qb : P * (qb + 1)],
                                         rhs=wo_sb[:, c, :], start=(c == 0), stop=(c == 3))
                    nc.vector.tensor_add(x2_sb[:, qb, :], po, xq_sb[:, qb, :])
                    st6 = st2.tile([P, 6], F32, tag="st6")
                    nc.vector.bn_stats(out=st6, in_=x2_sb[:, qb, :])
                    nc.vector.bn_aggr(out=mv2[:, jq, :], in_=st6)
                # exact 1/std(ddof=1) for LN2: rsqrt bit trick + 2 Newton
                # (this scale multiplies the large FFN branch)
                sc2 = st2.tile([P, 4], F32, tag="sc")
                tt2 = st2.tile([P, 4], F32, tag="tt")
                y0 = st2.tile([P, 4], F32, tag="y0")
                zz = st2.tile([P, 4], F32, tag="zz")
                ww = st2.tile([P, 4], F32, tag="ww")
                var2 = mv2[:, :, 1]
                nc.vector.tensor_scalar(out=y0.bitcast(I32), in0=var2.bitcast(I32),
                                        scalar1=-0.5, scalar2=RSQ_C,
                                        op0=ALU.mult, op1=ALU.add)
                for _ in range(2):
                    nc.vector.tensor_mul(zz, y0, y0)
                    nc.vector.tensor_mul(zz, zz, var2)
                    nc.vector.tensor_scalar(out=ww, in0=zz, scalar1=-0.5, scalar2=1.5,
                                            op0=ALU.mult, op1=ALU.add)
                    nc.vector.tensor_mul(y0, ww, y0)
                nc.vector.tensor_scalar(out=sc2, in0=y0, scalar1=ln_sc[:, 2:3],
                                        scalar2=float(1.0 / np.sqrt(K1)),
                                        op0=ALU.mult, op1=ALU.mult)
                nc.vector.tensor_mul(tt2, mv2[:, :, 0], sc2)
                nc.vector.tensor_scalar(out=tt2, in0=tt2, scalar1=ln_sc[:, 3:4], scalar2=None, op0=ALU.subtract)
                for jq in range(4):
                    qb = 4 * qc2 + jq
                    nc.vector.tensor_scalar(out=xn2[:, jq, :], in0=x2_sb[:, qb, :],
                                            scalar1=sc2[:, jq : jq + 1], scalar2=tt2[:, jq : jq + 1],
                                            op0=ALU.mult, op1=ALU.subtract)
                    ptp2 = pso.tile([P, 4, P], BF16, tag="po")
                    for e in range(4):
                        nc.tensor.transpose(ptp2[:, e, :], xn2[:, jq, P * e : P * (e + 1)], ident)
                    if jq % 2 == 0:
                        nc.scalar.copy(out=xn2T[:, :, P * qb : P * (qb + 1)], in_=ptp2)
                    else:
                        nc.vector.tensor_copy(out=xn2T[:, :, P * qb : P * (qb + 1)], in_=ptp2)

            # deferred work: the drain A@Vs, each head-pair's den/ctx
            # normalization, and each query-half's Wo/LN2 block are emitted
            # INSIDE the next iteration's kp loop, so no engine FIFO piles
            # up at iteration boundaries and the PE never idles long enough
            # for the HAM clock gate to re-throttle.
            pending = []
            norm_q = []
            wo_q = []
            for qc in range(2):
                qo = 512 * qc
                for hp in range(4):
                    ch = hp
                    pc_a = psc.tile([VW, 512], F32, tag="pc")
                    pc_b = psc.tile([VW, 512], F32, tag="pc")
                    pcs = [pc_a, pc_b]
                    for kp in range(NKP):
                        ep = expp.tile([P, 2, 2, 512], FP8E5, tag="est")
                        for j in range(2):
                            ki = 2 * kp + j
                            pool = pssa if j == 0 else pssb
                            ps = pool.tile([P, 2, 512], F32)
                            nc.tensor.matmul(ps[:, 0, :],
                                             lhsT=kT[0:64, ch, P * ki : P * (ki + 1)],
                                             rhs=qT[0:64, ch, qo : qo + 512],
                                             start=True, stop=True)
                            nc.tensor.matmul(ps[:, 1, :],
                                             lhsT=kT[64:128, ch, P * ki : P * (ki + 1)],
                                             rhs=qT[64:128, ch, qo : qo + 512],
                                             start=True, stop=True)
                            # whole-kp engine alternation: each est tile has
                            # a single writer, so ACT and DVE never serialize
                            # on a shared-tile WAW dependency
                            if kp % 2 == 0:
                                nc.scalar.activation(out=ep[:, j, :, :], in_=ps,
                                                     func=AF.Exp, scale=1.0 / 8.0)
                            else:
                                nc.vector.tensor_scalar(out=ep[:, j, :, :].bitcast(I8),
                                                        in0=ps, scalar1=A_SCH, scalar2=B_SCH,
                                                        op0=ALU.mult, op1=ALU.add)
                        pending.append((pcs, hp, ep, kp))
                        # A@V lags two k-tile pairs so its est operand is
                        # always long since finished
                        if len(pending) > 2:
                            av(*pending.pop(0))
                        if kp == 3 and norm_q:
                            flush_norm(*norm_q.pop(0))
                        if kp == 6 and wo_q:
                            flush_wo(wo_q.pop(0))
                        if fillers and kp < 10:
                            # 10 fillers per (hp,qc) iteration: all of chunk
                            # c's K/Q projections land during iteration c-1,
                            # before any hp=c score matmul needs them
                            fc_, ftb, fw, fdst, fbias = fillers.popleft()
                            kq_group(fc_, ftb, fw, fdst, fbias, pso)
                    norm_q.append((pcs, ch, qo))
                wo_q.append(qc)
            for item in pending:
                av(*item)
            for item in norm_q:
                flush_norm(*item)
            for q in wo_q:
                flush_wo(q)

        kqv_cm.__exit__(None, None, None)

        # ---- phase 4: FFN -----------------------------------------------
        with tc.tile_pool(name="p4", bufs=1) as p4, \
             tc.tile_pool(name="out_s", bufs=4) as outs, \
             tc.tile_pool(name="ps_h", bufs=2, space="PSUM") as psh, \
             tc.tile_pool(name="ps_f", bufs=2, space="PSUM") as psf:
            h1T = p4.tile([P, 16, QPC], BF16)
            for q2 in range(2):
                for fg in range(8):
                    ph = psh.tile([P, 2, 512], F32)
                    for fi in range(2):
                        fc = 2 * fg + fi
                        for e in range(4):
                            nc.tensor.matmul(ph[:, fi, :],
                                             lhsT=w1_sb[:, e, P * fc : P * (fc + 1)],
                                             rhs=xn2T[:, e, 512 * q2 : 512 * (q2 + 1)],
                                             start=(e == 0), stop=(e == 3))
                    for fi in range(2):
                        fc = 2 * fg + fi
                        nc.scalar.activation(out=h1T[:, fc, 512 * q2 : 512 * (q2 + 1)],
                                             in_=ph[:, fi, :], func=AF.Relu,
                                             bias=b1_sb[:, fc : fc + 1])
            for qb in range(8):
                pf = psf.tile([P, E], F32)
                for fc in range(16):
                    nc.tensor.matmul(pf, lhsT=h1T[:, fc, P * qb : P * (qb + 1)],
                                     rhs=w2_sb[:, fc, :], start=(fc == 0), stop=(fc == 15))
                ot = outs.tile([P, E], F32)
                nc.vector.tensor_add(ot, pf, x2_sb[:, qb, :])
                nc.vector.tensor_add(ot, ot, b2_sb)
                nc.sync.dma_start(out=ext["out"][P * qb : P * (qb + 1), :], in_=ot)


def _build():
    if "nc" in _CACHE:
        return _CACHE["nc"]
    nc = bacc.Bacc(None, target_bir_lowering=False)
    ext = {
        "xb": nc.dram_tensor("xb", [S, E], BF16, kind="ExternalInput"),
        "xq": nc.dram_tensor("xq", [P, 8, E], F32, kind="ExternalInput"),
        "wq": nc.dram_tensor("wq", [P, 4, E], FP8, kind="ExternalInput"),
        "wk": nc.dram_tensor("wk", [P, 4, E], FP8, kind="ExternalInput"),
        "wv": nc.dram_tensor("wv", [P, 4, E], FP8, kind="ExternalInput"),
        "wo": nc.dram_tensor("wo", [P, 4, E], BF16, kind="ExternalInput"),
        "w1": nc.dram_tensor("w1", [P, 4, FF], BF16, kind="ExternalInput"),
        "w2": nc.dram_tensor("w2", [P, 16, E], BF16, kind="ExternalInput"),
        "bq": nc.dram_tensor("bq", [P, 4], F32, kind="ExternalInput"),
        "b1": nc.dram_tensor("b1", [P, 16], F32, kind="ExternalInput"),
        "b2": nc.dram_tensor("b2", [E], F32, kind="ExternalInput"),
        "ident": nc.dram_tensor("ident", [P, P], BF16, kind="ExternalInput"),
        "a1": nc.dram_tensor("a1", [1], F32, kind="ExternalInput"),
        "c1": nc.dram_tensor("c1", [1], F32, kind="ExternalInput"),
        "a2": nc.dram_tensor("a2", [1], F32, kind="ExternalInput"),
        "c2": nc.dram_tensor("c2", [1], F32, kind="ExternalInput"),
        "out": nc.dram_tensor("out", [QPC, E], F32, kind="ExternalOutput"),
    }
    with tile.TileContext(nc) as tc:
        _emit(nc, tc, ext)
    nc.finalize()
    _CACHE["nc"] = nc
    return nc


def kernel(x, mask, Wq, bq, Wk, bk, Wv, bv, Wo, bo, W1, b1, W2, b2,
           alpha1, bias1, alpha2, bias2, **_kw):
    x = np.asarray(x, dtype=np.float32)
    mask = np.asarray(mask)
    if not np.all(mask != 0):
        raise NotImplementedError("kernel assumes an all-ones attention mask")

    bf = ml_dtypes.bfloat16
    f8 = ml_dtypes.float8_e4m3

    def chunked(w, dt=bf):
        # [R, F] -> [128, R//128, F]: partition-contiguous for trivial DMA
        w = np.asarray(w, np.float32).astype(dt)
        r, f = w.shape
        return np.ascontiguousarray(w.reshape(r // 128, 128, f).transpose(1, 0, 2))

    w_bf = {
        "wq": chunked(Wq, f8), "wk": chunked(Wk, f8), "wv": chunked(Wv, f8),
        "wo": chunked(Wo), "w1": chunked(W1), "w2": chunked(W2),
    }
    # bk shifts every key by a constant vector -> adds a per-query constant
    # to all scores -> exactly cancelled by softmax.  bv passes through
    # attention unchanged (softmax rows sum to 1): ctx = attn@V + bv, so
    # bv@Wo + bo is a constant row folded into the residual input here.
    fold = (np.asarray(bv, np.float32) @ np.asarray(Wo, np.float32)
            + np.asarray(bo, np.float32)).astype(np.float32)
    common = dict(w_bf)
    common.update({
        "bq": np.ascontiguousarray(np.asarray(bq, np.float32).reshape(4, P).T),
        "b1": np.ascontiguousarray(np.asarray(b1, np.float32).reshape(16, P).T),
        "b2": np.ascontiguousarray(np.asarray(b2, np.float32)),
        "ident": np.ascontiguousarray(np.eye(P, dtype=np.float32).astype(bf)),
        "a1": np.ascontiguousarray(np.asarray(alpha1, np.float32).reshape(1)),
        "c1": np.ascontiguousarray(np.asarray(bias1, np.float32).reshape(1)),
        "a2": np.ascontiguousarray(np.asarray(alpha2, np.float32).reshape(1)),
        "c2": np.ascontiguousarray(np.asarray(bias2, np.float32).reshape(1)),
    })

    in_maps = []
    for c in range(NCORES):
        b = c // 4
        qr = (c % 4) * QPC
        # rotate so this core's queries are tokens 0..QPC-1 (attention is
        # invariant to key/value ordering; mask is all ones)
        xb = np.concatenate([x[b, qr : qr + QPC], x[b, :qr], x[b, qr + QPC :]], axis=0)
        m = dict(common)
        m["xb"] = np.ascontiguousarray(xb.astype(bf))
        xqf = (x[b, qr : qr + QPC] + fold[None, :]).reshape(8, P, E).transpose(1, 0, 2)
        m["xq"] = np.ascontiguousarray(xqf)
        in_maps.append(m)

    nc = _build()
    res = run_bass_kernel_spmd(nc, in_maps, core_ids=list(range(NCORES)),
                               **_kw.get("_run_kwargs", {}))

    out = np.empty((B, S, E), dtype=np.float32)
    for c in range(NCORES):
        b = c // 4
        qr = (c % 4) * QPC
        out[b, qr : qr + QPC] = res.results[c]["out"]
    if _kw.get("_return_res"):
        return out, res
    return out


# revision 23
# speedup vs baseline: 1.1916x; 1.1916x over previous
"""Trainium2 Bass kernel for a pre-norm transformer encoder layer.

Problem shapes: B=2, S=4096, E=512, H=8 (Dh=64), FF=2048, fp32 I/O.

Sharding (zero cross-core communication): core c handles batch b=c//4 and
query rows qr=(c%4)*1024.  Each core redundantly computes LN1 + K/V for its
batch's full 4096 tokens, then attention for all 8 heads over its own 1024
queries, then Wo / LN2 / FFN token-parallel.  The per-core token stream is
rotated so the core's queries are tokens 0..1023 — attention is invariant
to key/value ordering, so one SPMD program serves all cores.

Engine strategy:
- softmax exp alternates between ScalarE (native Exp LUT -> fp8e5m2) and
  VectorE (Schraudolph bit trick: e5m2 bitpattern = round(4/ln2 * s/8 +
  60 - 0.23) via one tensor_scalar into an int8 view).  The ~6-12%
  sawtooth washes out after softmax averaging over ~1.5k effective keys.
- A@V runs in fp8 DoubleRow mode: V in e4m3 padded to 80 columns (ones at
  col 64 produce the softmax denominators; 65..79 zero), two k-tiles per
  matmul -> half the PE slots of the bf16 version.
- 1/den and LN1's 1/std come from a VectorE log bit-trick followed by a
  ScalarE exp (these errors only perturb the small attention branch);
  LN2's 1/std (which scales the large FFN branch) uses an exact rsqrt
  bit-trick + 2 Newton iterations on VectorE.  No Ln/Sqrt activations ->
  the whole kernel uses a single ACT table set (exp/relu/copy).
"""

import sys

if "/opt/trn_rl_repo" not in sys.path:
    sys.path.insert(0, "/opt/trn_rl_repo")

from contextlib import ExitStack

import ml_dtypes
import numpy as np

import concourse.bacc as bacc
import concourse.tile as tile
from concourse import mybir
from concourse.bass_utils import run_bass_kernel_spmd

B, S, E, H, Dh, FF = 2, 4096, 512, 8, 64, 2048
NCORES = 8
QPC = 1024  # queries per core
F32 = mybir.dt.float32
BF16 = mybir.dt.bfloat16
I8 = mybir.dt.int8
I32 = mybir.dt.int32
FP8 = mybir.dt.float8e4
FP8E5 = mybir.dt.float8e5
AF = mybir.ActivationFunctionType
ALU = mybir.AluOpType
DR = mybir.MatmulPerfMode.DoubleRow
P = 128
NKT = S // P   # 32 k-tiles
NKP = NKT // 2  # 16 k-tile pairs
VW = 80        # per-head V columns: 64 data + ones@64 + zero pad to 80

LN2_ = float(np.log(2.0))
A_SCH = 4.0 / (8.0 * LN2_)   # e5m2 Schraudolph slope (1/sqrt(Dh) folded)
B_SCH = 4.0 * 15.0 - 0.23
KLOG = 8.262958405e-8        # ln2 / 2^23
B0 = 1064866805.0            # fp32 log bit-trick bias (mu=0.0430)
K1 = float(E) / (E - 1)      # unbiased-variance correction
C_LN1 = B0 - float(np.log(K1)) / KLOG
RSQ_C = float(0x5F3759DF)

_CACHE = {}


def _emit(nc, tc, ext):
    es = ExitStack()
    with es:
        persist = es.enter_context(tc.tile_pool(name="persist", bufs=1))
        p34 = es.enter_context(tc.tile_pool(name="p34", bufs=1))
        st2 = es.enter_context(tc.tile_pool(name="st2", bufs=4))
        kqv_cm = tc.tile_pool(name="kqv", bufs=1)
        kqv = kqv_cm.__enter__()

        xq_sb = persist.tile([P, 8, E], F32)
        x2_sb = persist.tile([P, 8, E], F32)
        ctxT = persist.tile([P, 4, QPC], BF16)
        bq_sb = persist.tile([P, 4], F32)
        b1_sb = persist.tile([P, 16], F32)
        b2_sb = persist.tile([P, E], F32)
        ln_sc = persist.tile([P, 4], F32)  # alpha1,bias1,alpha2,bias2 bcast
        ident = persist.tile([P, P], BF16)
        xn2T = p34.tile([P, 4, QPC], BF16)
        xn2 = p34.tile([P, 4, E], BF16)
        wo_sb = p34.tile([P, 4, E], BF16)
        w1_sb = persist.tile([P, 4, FF], BF16)
        w2_sb = persist.tile([P, 16, E], BF16)

        kT = kqv.tile([P, 4, S], BF16)
        qT = kqv.tile([P, 4, QPC], BF16)
        wq_sb = kqv.tile([P, 4, E], FP8)
        wk_sb = kqv.tile([P, 4, E], FP8)
        xnT = kqv.tile([P, 4, S], FP8)
        vD = kqv.tile([P, NKP, H, 2, VW], FP8)

        # ---- setup loads -------------------------------------------------
        nc.sync.dma_start(out=bq_sb, in_=ext["bq"][:])
        nc.gpsimd.dma_start(out=b2_sb, in_=ext["b2"][:].unsqueeze(0).to_broadcast((P, E)))
        for i, nm in enumerate(["a1", "c1", "a2", "c2"]):
            nc.gpsimd.dma_start(out=ln_sc[:, i : i + 1], in_=ext[nm][:].to_broadcast((P, 1)))
        nc.sync.dma_start(out=ident, in_=ext["ident"][:])
        nc.vector.memset(vD[:, :, :, :, Dh : Dh + 1], 1.0)
        nc.vector.memset(vD[:, :, :, :, Dh + 1 :], 0.0)

        # ---- phase 0/1: LN1, transpose, QKV projections ------------------
        with tc.tile_pool(name="wqkv", bufs=1) as wp, \
             tc.tile_pool(name="xn_s", bufs=3) as xnp, \
             tc.tile_pool(name="x_s", bufs=12) as xs, \
             tc.tile_pool(name="st1", bufs=6) as stp, \
             tc.tile_pool(name="ps1", bufs=4, space="PSUM") as ps1:

            wv_sb = wp.tile([P, 4, E], FP8)

            xtiles = []
            for i in range(NKT):
                xt = xs.tile([P, E], BF16)
                eng = nc.sync if i < 12 else nc.gpsimd
                eng.dma_start(out=xt, in_=ext["xb"][P * i : P * (i + 1), :])
                xtiles.append(xt)
                if i == 3:
                    nc.sync.dma_start(out=wk_sb, in_=ext["wk"][:])
                    nc.sync.dma_start(out=wv_sb, in_=ext["wv"][:])
                    nc.sync.dma_start(out=wq_sb, in_=ext["wq"][:])
                if i == 11:
                    nc.sync.dma_start(out=xq_sb, in_=ext["xq"][:])
                    nc.sync.dma_start(out=b1_sb, in_=ext["b1"][:])
                    nc.gpsimd.dma_start(out=wo_sb, in_=ext["wo"][:])
                    # prefetch FFN weights so the tail never waits on DMA
                    nc.gpsimd.dma_start(out=w1_sb, in_=ext["w1"][:])
                    nc.gpsimd.dma_start(out=w2_sb, in_=ext["w2"][:])

            def kq_group(c, tb, w_sb, dstT, bias, pool):
                # fp8 DoubleRow: two k-tile-paired matmuls instead of four;
                # PSUM evacuation on ScalarE (VectorE is the busier engine)
                acc = pool.tile([P, E], F32, tag="po")
                for p in range(2):
                    nc.tensor.matmul(acc, lhsT=w_sb[:, 2 * p : 2 * p + 2, P * c : P * (c + 1)],
                                     rhs=xnT[:, 2 * p : 2 * p + 2, 512 * tb : 512 * (tb + 1)],
                                     start=(p == 0), stop=(p == 1), perf_mode=DR)
                dst = dstT[:, c, 512 * tb : 512 * (tb + 1)]
                if bias is None:
                    nc.scalar.copy(out=dst, in_=acc)
                else:
                    nc.scalar.activation(out=dst, in_=acc, func=AF.Identity,
                                         bias=bias[:, c : c + 1])

            for g in range(8):
                mv = stp.tile([P, 4, 2], F32, tag="mv")
                for j in range(4):
                    i = 4 * g + j
                    st6 = stp.tile([P, 6], F32, tag="st6")
                    nc.vector.bn_stats(out=st6, in_=xtiles[i])
                    nc.vector.bn_aggr(out=mv[:, j, :], in_=st6)
                sc = stp.tile([P, 4], F32, tag="sc")
                tt = stp.tile([P, 4], F32, tag="tt")
                y1 = stp.tile([P, 4], F32, tag="y1")
                z1 = stp.tile([P, 4], F32, tag="z1")
                # 1/std(ddof=1): rsqrt bit trick + 1 Newton (max ~0.2% err)
                var1 = mv[:, :, 1]
                nc.vector.tensor_scalar(out=y1.bitcast(I32), in0=var1.bitcast(I32),
                                        scalar1=-0.5, scalar2=RSQ_C,
                                        op0=ALU.mult, op1=ALU.add)
                nc.vector.tensor_mul(z1, y1, y1)
                nc.vector.tensor_mul(z1, z1, var1)
                nc.vector.tensor_scalar(out=z1, in0=z1, scalar1=-0.5, scalar2=1.5,
                                        op0=ALU.mult, op1=ALU.add)
                nc.vector.tensor_mul(y1, z1, y1)
                # sc = rsqrt(var)*alpha1/sqrt(K1)
                nc.vector.tensor_scalar(out=sc, in0=y1, scalar1=ln_sc[:, 0:1],
                                        scalar2=float(1.0 / np.sqrt(K1)),
                                        op0=ALU.mult, op1=ALU.mult)
                nc.vector.tensor_mul(tt, mv[:, :, 0], sc)
                nc.vector.tensor_scalar(out=tt, in0=tt, scalar1=ln_sc[:, 1:2], scalar2=None, op0=ALU.subtract)
                for j in range(4):
                    i = 4 * g + j
                    xnt = xnp.tile([P, E], BF16)
                    nc.vector.tensor_scalar(out=xnt, in0=xtiles[i], scalar1=sc[:, j : j + 1],
                                            scalar2=tt[:, j : j + 1], op0=ALU.mult, op1=ALU.subtract)
                    ptp = ps1.tile([P, 4, P], BF16, tag="ptp")
                    for e in range(4):
                        nc.tensor.transpose(ptp[:, e, :], xnt[:, P * e : P * (e + 1)], ident)
                    nc.scalar.copy(out=xnT[:, :, P * i : P * (i + 1)], in_=ptp)
                # this token block is transposed: K/Q chunk-0 + its V k-tiles
                kq_group(0, g, wk_sb, kT, None, ps1)
                if g < 2:
                    kq_group(0, g, wq_sb, qT, bq_sb, ps1)
                for j in range(4):
                    kt = 4 * g + j
                    acc = ps1.tile([P, E], F32, tag="po")
                    for p in range(2):
                        nc.tensor.matmul(acc, lhsT=xnT[:, 2 * p : 2 * p + 2, P * kt : P * (kt + 1)],
                                         rhs=wv_sb[:, 2 * p : 2 * p + 2, :],
                                         start=(p == 0), stop=(p == 1), perf_mode=DR)
                    dst = vD[:, kt // 2, :, kt % 2, 0:Dh]
                    nc.scalar.copy(out=dst, in_=acc.rearrange("p (h d) -> p h d", d=Dh))

        # ---- phase 2: attention (+ overlapped Wo/LN2 per query half) ----
        with tc.tile_pool(name="exp_p", bufs=4) as expp, \
             tc.tile_pool(name="rs_p", bufs=4) as rsp, \
             tc.tile_pool(name="ps_sa", bufs=1, space="PSUM") as pssa, \
             tc.tile_pool(name="ps_sb", bufs=1, space="PSUM") as pssb, \
             tc.tile_pool(name="ps_c", bufs=2, space="PSUM") as psc, \
             tc.tile_pool(name="ps_o", bufs=2, space="PSUM") as pso:
            from collections import deque
            fillers = deque()
            for c in range(1, 4):
                for tb in range(8):
                    fillers.append((c, tb, wk_sb, kT, None))
                for tb in range(2):
                    fillers.append((c, tb, wq_sb, qT, bq_sb))
            def av(pcs, hp2, pep, pkp):
                for par in range(2):
                    nc.tensor.matmul(pcs[par][:, :],
                                     lhsT=vD[:, pkp, 2 * hp2 + par, :, :],
                                     rhs=pep[:, :, par, :],
                                     start=(pkp == 0), stop=(pkp == NKP - 1),
                                     perf_mode=DR)

            def flush_norm(pcs, ch2, qo2):
                for par in range(2):
                    h = 2 * ch2 + par
                    r0 = 64 * (h % 2)
                    # 1/den = exp(-ln(den)), ln via bit trick on DVE
                    lnd = rsp.tile([1, 512], F32, tag="lnd")
                    rs = rsp.tile([1, 512], F32, tag="rs")
                    nc.vector.tensor_scalar(out=lnd, in0=pcs[par][Dh : Dh + 1, :].bitcast(I32),
                                            scalar1=B0, scalar2=-KLOG,
                                            op0=ALU.subtract, op1=ALU.mult)
                    nc.scalar.activation(out=rs, in_=lnd, func=AF.Exp)
                    bc = rsp.tile([64, 512], F32, tag="bc")
                    nc.gpsimd.partition_broadcast(bc, rs)
                    nc.vector.tensor_mul(ctxT[r0 : r0 + 64, ch2, qo2 : qo2 + 512],
                                         pcs[par][0:Dh, :], bc)

            def flush_wo(qc2):
                # Wo + residual + LN2 + xn2 transpose for one query half
                mv2 = st2.tile([P, 4, 2], F32, tag="mv")
                for jq in range(4):
                    qb = 4 * qc2 + jq
                    po = pso.tile([P, E], F32, tag="po")
                    for c in range(4):
                        nc.tensor.matmul(po, lhsT=ctxT[:, c, P * qb : P * (qb + 1)],
                                         rhs=wo_sb[:, c, :], start=(c == 0), stop=(c == 3))
                    nc.vector.tensor_add(x2_sb[:, qb, :], po, xq_sb[:, qb, :])
                    st6 = st2.tile([P, 6], F32, tag="st6")
                    nc.vector.bn_stats(out=st6, in_=x2_sb[:, qb, :])
                    nc.vector.bn_aggr(out=mv2[:, jq, :], in_=st6)
                # exact 1/std(ddof=1) for LN2: rsqrt bit trick + 2 Newton
                # (this scale multiplies the large FFN branch)
                sc2 = st2.tile([P, 4], F32, tag="sc")
                tt2 = st2.tile([P, 4], F32, tag="tt")
                y0 = st2.tile([P, 4], F32, tag="y0")
                zz = st2.tile([P, 4], F32, tag="zz")
                ww = st2.tile([P, 4], F32, tag="ww")
                var2 = mv2[:, :, 1]
                nc.vector.tensor_scalar(out=y0.bitcast(I32), in0=var2.bitcast(I32),
                                        scalar1=-0.5, scalar2=RSQ_C,
                                        op0=ALU.mult, op1=ALU.add)
                for _ in range(2):
                    nc.vector.tensor_mul(zz, y0, y0)
                    nc.vector.tensor_mul(zz, zz, var2)
                    nc.vector.tensor_scalar(out=ww, in0=zz, scalar1=-0.5, scalar2=1.5,
                                            op0=ALU.mult, op1=ALU.add)
                    nc.vector.tensor_mul(y0, ww, y0)
                nc.vector.tensor_scalar(out=sc2, in0=y0, scalar1=ln_sc[:, 2:3],
                                        scalar2=float(1.0 / np.sqrt(K1)),
                                        op0=ALU.mult, op1=ALU.mult)
                nc.vector.tensor_mul(tt2, mv2[:, :, 0], sc2)
                nc.vector.tensor_scalar(out=tt2, in0=tt2, scalar1=ln_sc[:, 3:4], scalar2=None, op0=ALU.subtract)
                for jq in range(4):
                    qb = 4 * qc2 + jq
                    nc.vector.tensor_scalar(out=xn2[:, jq, :], in0=x2_sb[:, qb, :],
                                            scalar1=sc2[:, jq : jq + 1], scalar2=tt2[:, jq : jq + 1],
                                            op0=ALU.mult, op1=ALU.subtract)
                    ptp2 = pso.tile([P, 4, P], BF16, tag="po")
                    for e in range(4):
                        nc.tensor.transpose(ptp2[:, e, :], xn2[:, jq, P * e : P * (e + 1)], ident)
                    if jq % 2 == 0:
                        nc.scalar.copy(out=xn2T[:, :, P * qb : P * (qb + 1)], in_=ptp2)
                    else:
                        nc.vector.tensor_copy(out=xn2T[:, :, P * qb : P * (qb + 1)], in_=ptp2)

            # deferred work: the drain A@Vs, each head-pair's den/ctx
            # normalization, and each query-half's Wo/LN2 block are emitted
            # INSIDE the next iteration's kp loop, so no engine FIFO piles
            # up at iteration boundaries and the PE never idles long enough
            # for the HAM clock gate to re-throttle.
            pending = []
            norm_q = []
            wo_q = []
            for qc in range(2):
                qo = 512 * qc
                for hp in range(4):
                    ch = hp
                    pc_a = psc.tile([VW, 512], F32, tag="pc")
                    pc_b = psc.tile([VW, 512], F32, tag="pc")
                    pcs = [pc_a, pc_b]
                    for kp in range(NKP):
                        ep = expp.tile([P, 2, 2, 512], FP8E5, tag="est")
                        for j in range(2):
                            ki = 2 * kp + j
                            pool = pssa if j == 0 else pssb
                            ps = pool.tile([P, 2, 512], F32)
                            nc.tensor.matmul(ps[:, 0, :],
                                             lhsT=kT[0:64, ch, P * ki : P * (ki + 1)],
                                             rhs=qT[0:64, ch, qo : qo + 512],
                                             start=True, stop=True)
                            nc.tensor.matmul(ps[:, 1, :],
                                             lhsT=kT[64:128, ch, P * ki : P * (ki + 1)],
                                             rhs=qT[64:128, ch, qo : qo + 512],
                                             start=True, stop=True)
                            # whole-kp engine alternation: each est tile has
                            # a single writer, so ACT and DVE never serialize
                            # on a shared-tile WAW dependency
                            if kp % 2 == 0:
                                nc.scalar.activation(out=ep[:, j, :, :], in_=ps,
                                                     func=AF.Exp, scale=1.0 / 8.0)
                            else:
                                nc.vector.tensor_scalar(out=ep[:, j, :, :].bitcast(I8),
                                                        in0=ps, scalar1=A_SCH, scalar2=B_SCH,
                                                        op0=ALU.mult, op1=ALU.add)
                        pending.append((pcs, hp, ep, kp))
                        # A@V lags two k-tile pairs so its est operand is
                        # always long since finished
                        if len(pending) > 2:
                            av(*pending.pop(0))
                        if kp == 3 and norm_q:
                            flush_norm(*norm_q.pop(0))
                        if kp == 6 and wo_q:
                            flush_wo(wo_q.pop(0))
                        if fillers and kp < 10:
                            # 10 fillers per (hp,qc) iteration: all of chunk
                            # c's K/Q projections land during iteration c-1,
                            # before any hp=c score matmul needs them
                            fc_, ftb, fw, fdst, fbias = fillers.popleft()
                            kq_group(fc_, ftb, fw, fdst, fbias, pso)
                    norm_q.append((pcs, ch, qo))
                wo_q.append(qc)
            for item in pending:
                av(*item)
            for item in norm_q:
                flush_norm(*item)
            for q in wo_q:
                flush_wo(q)

        kqv_cm.__exit__(None, None, None)

        # ---- phase 4: FFN -----------------------------------------------
        with tc.tile_pool(name="p4", bufs=1) as p4, \
             tc.tile_pool(name="out_s", bufs=4) as outs, \
             tc.tile_pool(name="ps_h", bufs=2, space="PSUM") as psh, \
             tc.tile_pool(name="ps_f", bufs=2, space="PSUM") as psf:
            h1T = p4.tile([P, 16, QPC], BF16)
            for q2 in range(2):
                for fg in range(8):
                    ph = psh.tile([P, 2, 512], F32)
                    for fi in range(2):
                        fc = 2 * fg + fi
                        for e in range(4):
                            nc.tensor.matmul(ph[:, fi, :],
                                             lhsT=w1_sb[:, e, P * fc : P * (fc + 1)],
                                             rhs=xn2T[:, e, 512 * q2 : 512 * (q2 + 1)],
                                             start=(e == 0), stop=(e == 3))
                    for fi in range(2):
                        fc = 2 * fg + fi
                        nc.scalar.activation(out=h1T[:, fc, 512 * q2 : 512 * (q2 + 1)],
                                             in_=ph[:, fi, :], func=AF.Relu,
                                             bias=b1_sb[:, fc : fc + 1])
            for qb in range(8):
                pf = psf.tile([P, E], F32)
                for fc in range(16):
                    nc.tensor.matmul(pf, lhsT=h1T[:, fc, P * qb : P * (qb + 1)],
                                     rhs=w2_sb[:, fc, :], start=(fc == 0), stop=(fc == 15))
                ot = outs.tile([P, E], F32)
                nc.vector.tensor_add(ot, pf, x2_sb[:, qb, :])
                nc.vector.tensor_add(ot, ot, b2_sb)
                nc.sync.dma_start(out=ext["out"][P * qb : P * (qb + 1), :], in_=ot)


def _build():
    if "nc" in _CACHE:
        return _CACHE["nc"]
    nc = bacc.Bacc(None, target_bir_lowering=False)
    ext = {
        "xb": nc.dram_tensor("xb", [S, E], BF16, kind="ExternalInput"),
        "xq": nc.dram_tensor("xq", [P, 8, E], F32, kind="ExternalInput"),
        "wq": nc.dram_tensor("wq", [P, 4, E], FP8, kind="ExternalInput"),
        "wk": nc.dram_tensor("wk", [P, 4, E], FP8, kind="ExternalInput"),
        "wv": nc.dram_tensor("wv", [P, 4, E], FP8, kind="ExternalInput"),
        "wo": nc.dram_tensor("wo", [P, 4, E], BF16, kind="ExternalInput"),
        "w1": nc.dram_tensor("w1", [P, 4, FF], BF16, kind="ExternalInput"),
        "w2": nc.dram_tensor("w2", [P, 16, E], BF16, kind="ExternalInput"),
        "bq": nc.dram_tensor("bq", [P, 4], F32, kind="ExternalInput"),
        "b1": nc.dram_tensor("b1", [P, 16], F32, kind="ExternalInput"),
        "b2": nc.dram_tensor("b2", [E], F32, kind="ExternalInput"),
        "ident": nc.dram_tensor("ident", [P, P], BF16, kind="ExternalInput"),
        "a1": nc.dram_tensor("a1", [1], F32, kind="ExternalInput"),
        "c1": nc.dram_tensor("c1", [1], F32, kind="ExternalInput"),
        "a2": nc.dram_tensor("a2", [1], F32, kind="ExternalInput"),
        "c2": nc.dram_tensor("c2", [1], F32, kind="ExternalInput"),
        "out": nc.dram_tensor("out", [QPC, E], F32, kind="ExternalOutput"),
    }
    with tile.TileContext(nc) as tc:
        _emit(nc, tc, ext)
    nc.finalize()
    _CACHE["nc"] = nc
    return nc


def kernel(x, mask, Wq, bq, Wk, bk, Wv, bv, Wo, bo, W1, b1, W2, b2,
           alpha1, bias1, alpha2, bias2, **_kw):
    x = np.asarray(x, dtype=np.float32)
    mask = np.asarray(mask)
    if not np.all(mask != 0):
        raise NotImplementedError("kernel assumes an all-ones attention mask")

    bf = ml_dtypes.bfloat16
    f8 = ml_dtypes.float8_e4m3

    def chunked(w, dt=bf):
        # [R, F] -> [128, R//128, F]: partition-contiguous for trivial DMA
        w = np.asarray(w, np.float32).astype(dt)
        r, f = w.shape
        return np.ascontiguousarray(w.reshape(r // 128, 128, f).transpose(1, 0, 2))

    w_bf = {
        "wq": chunked(Wq, f8), "wk": chunked(Wk, f8), "wv": chunked(Wv, f8),
        "wo": chunked(Wo), "w1": chunked(W1), "w2": chunked(W2),
    }
    # bk shifts every key by a constant vector -> adds a per-query constant
    # to all scores -> exactly cancelled by softmax.  bv passes through
    # attention unchanged (softmax rows sum to 1): ctx = attn@V + bv, so
    # bv@Wo + bo is a constant row folded into the residual input here.
    fold = (np.asarray(bv, np.float32) @ np.asarray(Wo, np.float32)
            + np.asarray(bo, np.float32)).astype(np.float32)
    common = dict(w_bf)
    common.update({
        "bq": np.ascontiguousarray(np.asarray(bq, np.float32).reshape(4, P).T),
        "b1": np.ascontiguousarray(np.asarray(b1, np.float32).reshape(16, P).T),
        "b2": np.ascontiguousarray(np.asarray(b2, np.float32)),
        "ident": np.ascontiguousarray(np.eye(P, dtype=np.float32).astype(bf)),
        "a1": np.ascontiguousarray(np.asarray(alpha1, np.float32).reshape(1)),
        "c1": np.ascontiguousarray(np.asarray(bias1, np.float32).reshape(1)),
        "a2": np.ascontiguousarray(np.asarray(alpha2, np.float32).reshape(1)),
        "c2": np.ascontiguousarray(np.asarray(bias2, np.float32).reshape(1)),
    })

    in_maps = []
    for c in range(NCORES):
        b = c // 4
        qr = (c % 4) * QPC
        # rotate so this core's queries are tokens 0..QPC-1 (attention is
        # invariant to key/value ordering; mask is all ones)
        xb = np.concatenate([x[b, qr : qr + QPC], x[b, :qr], x[b, qr + QPC :]], axis=0)
        m = dict(common)
        m["xb"] = np.ascontiguousarray(xb.astype(bf))
        xqf = (x[b, qr : qr + QPC] + fold[None, :]).reshape(8, P, E).transpose(1, 0, 2)
        m["xq"] = np.ascontiguousarray(xqf)
        in_maps.append(m)

    nc = _build()
    res = run_bass_kernel_spmd(nc, in_maps, core_ids=list(range(NCORES)),
                               **_kw.get("_run_kwargs", {}))

    out = np.empty((B, S, E), dtype=np.float32)
    for c in range(NCORES):
        b = c // 4
        qr = (c % 4) * QPC
        out[b, qr : qr + QPC] = res.results[c]["out"]
    if _kw.get("_return_res"):
        return out, res
    return out
